# revision 11
# baseline (speedup 1.0000x reference)
"""MixConv kernel for Trainium2 (Bass/Tile), data-parallel over batch on 8 NeuronCores.

Reference computation (per sample b):
    mix[b]    = lat[b] @ w_dyn.T + b_dyn                      # [NMIX]
    kern[b]   = sum_m mix[b,m] * kernel_mix[m]                # [FOUT, FIN]
    bias[b]   = sum_m mix[b,m] * bias_mix[m]                  # [FOUT]
    out[b]    = kern[b] @ x[b].reshape(FIN, H*W) + bias[b][:, None]

Sharding: batch 16 -> 2 samples per core x 8 cores.

The kernel is HBM-bound (~358 GB/s per NeuronCore when all cores are
active), so the streamed tensors are narrowed on the host (kernel()
receives full fp32; casts are host-side preprocessing): out is written
bf16 and upcast on the host, and x streams as a per-channel precision
split -- the first NB input channels in bf16, the last NF8 in fp8-e3m4.
PSUM accumulation stays fp32 and the per-sample kernel weights stay bf16.
The max-norm error is dominated by worst-case quantization events, so it
plateaus in the fp8 channel count (12/16 fp8 ~= 16/16 fp8 in sim) while
read bytes drop another 37.5% vs all-bf16.  NF8=0 falls back to the pure
bf16 stream (rel-err 4.4e-3; split: ~1.6e-2, gate 2e-2).

The tiny dynamic-weight computation (mix/kern/bias: ~65K FLOPs on 16x512
inputs) is folded on the host into ready-made per-core tensors, so the
device-side setup is just three small DMA loads and the streaming matmuls
start immediately:
  lhsT[p=(s,i,j), q=(s',o,j')] = kern_s[o,i] if s==s' and j==j' else 0
  biasv[q=(s,o,j)] = bias_s[o]
lhsT is row-split to match the channel split; each 512-column chunk is
two accumulating matmuls (bf16 lhsT x bf16 rhs, then bf16 lhsT x fp8 rhs).

Per-core layout (driven by DMA bandwidth: only pure-2D [rows, nt] access
patterns sustain line rate; any 3-dim AP drops to ~100-160GB/s):
  xb viewed as [S*NB*NJ, CHW]  with partition (s, i<NB, j)   (C order)
  x8 viewed as [S*NF8*NJ, CHW] with partition (s, i>=NB, j)  (C order)
  out viewed as [128, CHW]     with partition q = (s, o, j)  (C order)
  One PSUM accumulation group per 512 columns (PSUM bank limit); bias is
  added during the PSUM->SBUF copy (alternating scalar/vector engines)
  which also converts fp32 -> bf16.  x loads ride the sync HWDGE ring,
  out stores the scalar(ACT) ring, 4-deep double-buffered.
"""

import numpy as np
import ml_dtypes

import concourse.bass as bass
import concourse.bacc as bacc
import concourse.tile as tile
import concourse.mybir as mybir
from concourse import bass_utils

B, FIN, FOUT, H, W = 16, 16, 16, 384, 384
LAT, NMIX = 512, 8
N_CORES = 8
S = B // N_CORES          # samples per core = 2
NJ = 4                    # HW chunks per sample
HW = H * W                # 147456
CHW = HW // NJ            # 36864
P = S * NJ * FIN          # 128 partitions
F32 = mybir.dt.float32
BF16 = mybir.dt.bfloat16
E3 = mybir.dt.float8e3

O_DT = BF16               # dtype out is streamed in (host-upcast)
X_NP = ml_dtypes.bfloat16
K_NP = ml_dtypes.bfloat16
E3_NP = ml_dtypes.float8_e3m4

NF8 = 12                  # input channels streamed as fp8-e3m4 (0 = all bf16)
NB = FIN - NF8            # input channels streamed as bf16

TILES = (9216,) * 4       # stream-tile columns; sums to CHW
assert sum(TILES) == CHW


def host_weights(lat, kernel_mix, bias_mix, w_dyn, b_dyn, nf8=NF8):
    """Per-core row-split block-diagonal lhsT (bf16) and bias vec (f32).

    Returns lists over cores: (lhsT_b [S*(FIN-nf8)*NJ, P], lhsT_8
    [S*nf8*NJ, P], biasv [P, 1]).  lhsT rows follow the (s, i, j) partition
    order of the corresponding x stream; lhsT_8 is None when nf8 == 0.
    """
    nb = FIN - nf8
    mix = lat @ w_dyn.T + b_dyn[None, :]                  # [B, NMIX]
    kern = np.einsum('bm,moi->boi', mix, kernel_mix)      # [B, FOUT, FIN]
    bias = np.einsum('bm,mo->bo', mix, bias_mix)          # [B, FOUT]
    lhsTbs, lhsT8s, biasvs = [], [], []
    i_idx = np.arange(FIN)
    o_idx = np.arange(FOUT)
    for c in range(N_CORES):
        lhsT = np.zeros((P, P), dtype=np.float32)
        biasv = np.zeros((P, 1), dtype=np.float32)
        for s in range(S):
            b = c * S + s
            for j in range(NJ):
                rows = s * FIN * NJ + i_idx * NJ + j      # p = (s, i, j)
                cols = s * FOUT * NJ + o_idx * NJ + j     # q = (s, o, j)
                lhsT[np.ix_(rows, cols)] = kern[b].T      # [i, o]
                biasv[cols, 0] = bias[b]
        lr = lhsT.reshape(S, FIN, NJ, P)
        lhsTbs.append(np.ascontiguousarray(
            lr[:, :nb].reshape(S * nb * NJ, P)).astype(K_NP))
        lhsT8s.append(np.ascontiguousarray(
            lr[:, nb:].reshape(S * nf8 * NJ, P)).astype(K_NP) if nf8 else None)
        biasvs.append(biasv)
    return lhsTbs, lhsT8s, biasvs


def make_in_maps(x, lat, kernel_mix, bias_mix, w_dyn, b_dyn, nf8=NF8):
    """Shard + host-precondition full fp32 inputs into per-core input maps."""
    nb = FIN - nf8
    x = np.ascontiguousarray(np.asarray(x, dtype=np.float32))
    lhsTbs, lhsT8s, biasvs = host_weights(
        np.asarray(lat, dtype=np.float32),
        np.asarray(kernel_mix, dtype=np.float32),
        np.asarray(bias_mix, dtype=np.float32),
        np.asarray(w_dyn, dtype=np.float32),
        np.asarray(b_dyn, dtype=np.float32), nf8=nf8)
    in_maps = []
    for c in range(N_CORES):
        xs = x[c * S:(c + 1) * S].reshape(S, FIN, NJ, CHW)
        m = {
            "xb": np.ascontiguousarray(xs[:, :nb]).astype(X_NP),
            "lhsTb": lhsTbs[c],
            "biasv": biasvs[c],
        }
        if nf8:
            m["x8"] = np.ascontiguousarray(xs[:, nb:]).astype(E3_NP)
            m["lhsT8"] = lhsT8s[c]
        in_maps.append(m)
    return in_maps


def build_nc(s=S, nj=NJ, chw=CHW, tiles=TILES, fin=FIN, fout=FOUT,
             n_cores=N_CORES, loop_repeat=1, mode="full", nf8=NF8,
             xs_bufs=4, os_bufs=4, ps_bufs=4, chunk=512,
             loop_covers_setup=False):
    p = s * nj * fin
    nb = fin - nf8
    pb = s * nb * nj
    p8 = s * nf8 * nj
    assert p <= 128 and sum(tiles) == chw
    assert all(t % chunk == 0 for t in tiles)
    ntmax = max(tiles)

    nc = bacc.Bacc("TRN2", target_bir_lowering=False, debug=False,
                   num_devices=n_cores)
    xb_d = nc.dram_tensor("xb", [s, nb, nj, chw], BF16, kind="ExternalInput").ap()
    lhsTb_d = nc.dram_tensor("lhsTb", [pb, p], BF16, kind="ExternalInput").ap()
    biasv_d = nc.dram_tensor("biasv", [p, 1], F32, kind="ExternalInput").ap()
    if nf8:
        x8_d = nc.dram_tensor("x8", [s, nf8, nj, chw], E3, kind="ExternalInput").ap()
        lhsT8_d = nc.dram_tensor("lhsT8", [p8, p], BF16, kind="ExternalInput").ap()
    out_d = nc.dram_tensor("out", [s, fout, nj, chw], O_DT, kind="ExternalOutput").ap()

    xbf = xb_d.rearrange("s i j c -> (s i j) c")    # [pb, chw], 2D
    x8f = x8_d.rearrange("s i j c -> (s i j) c") if nf8 else None
    of = out_d.rearrange("s o j c -> (s o j) c")    # [p, chw], 2D

    with tile.TileContext(nc) as tc:
        with (
            tc.tile_pool(name="setup", bufs=1) as setup,
            tc.tile_pool(name="xs", bufs=xs_bufs) as xs_pool,
            tc.tile_pool(name="x8s", bufs=xs_bufs) as x8_pool,
            tc.tile_pool(name="os", bufs=os_bufs) as os_pool,
            tc.tile_pool(name="ps", bufs=ps_bufs, space="PSUM") as ps_pool,
        ):
            def emit_setup():
                # Ready-made weights: small loads on the scalar(ACT) ring so
                # the sync ring starts streaming x immediately.
                lhsTb_sb = setup.tile([pb, p], BF16)
                nc.scalar.dma_start(out=lhsTb_sb[:], in_=lhsTb_d[:])
                lhsT8_sb = None
                if nf8:
                    lhsT8_sb = setup.tile([p8, p], BF16)
                    nc.scalar.dma_start(out=lhsT8_sb[:], in_=lhsT8_d[:])
                bias_sb = setup.tile([p, 1], F32)
                nc.scalar.dma_start(out=bias_sb[:], in_=biasv_d[:])
                return lhsTb_sb, lhsT8_sb, bias_sb

            def main_pass(lhsTb_sb, lhsT8_sb, bias_sb):
                col0 = 0
                for nt in tiles:
                    cols = slice(col0, col0 + nt)
                    col0 += nt
                    xbt = xs_pool.tile([pb, ntmax], BF16)
                    x8t = (x8_pool.tile([p8, ntmax], E3, name="x8t")
                           if nf8 else None)
                    if mode != "compute":
                        nc.sync.dma_start(out=xbt[:, :nt], in_=xbf[:, cols])
                        if nf8:
                            nc.sync.dma_start(out=x8t[:, :nt], in_=x8f[:, cols])
                    ot = os_pool.tile([p, ntmax], O_DT)
                    if mode != "dma":
                        for ci in range(nt // chunk):
                            cs = slice(ci * chunk, (ci + 1) * chunk)
                            pt = ps_pool.tile([p, chunk], F32)
                            nc.tensor.matmul(pt[:], lhsTb_sb[:], xbt[:, cs],
                                             start=True, stop=not nf8)
                            if nf8:
                                nc.tensor.matmul(pt[:], lhsT8_sb[:], x8t[:, cs],
                                                 start=False, stop=True)
                            if ci % 2 == 0:
                                nc.scalar.add(ot[:, cs], pt[:], bias_sb[:])
                            else:
                                nc.vector.tensor_scalar_add(ot[:, cs], pt[:],
                                                            bias_sb[:])
                    if mode != "compute":
                        # mode=="dma" stores the (unwritten) ot tile: pure
                        # load+store floor probe without compute deps.
                        nc.scalar.dma_start(out=of[:, cols], in_=ot[:, :nt])

            if loop_repeat > 1 and loop_covers_setup:
                with tc.For_i(0, loop_repeat, 1):
                    args = emit_setup()
                    main_pass(*args)
            elif loop_repeat > 1:
                args = emit_setup()
                with tc.For_i(0, loop_repeat, 1):
                    main_pass(*args)
            else:
                args = emit_setup()
                main_pass(*args)
    nc.compile()
    return nc


_NC = None


def _get_nc():
    global _NC
    if _NC is None:
        _NC = build_nc()
    return _NC


def kernel(x, lat, kernel_mix, bias_mix, w_dyn, b_dyn):
    in_maps = make_in_maps(x, lat, kernel_mix, bias_mix, w_dyn, b_dyn)
    nc = _get_nc()
    res = bass_utils.run_bass_kernel_spmd(nc, in_maps, core_ids=list(range(N_CORES)))
    out = np.empty((B, FOUT, H, W), dtype=np.float32)
    for c in range(N_CORES):
        out[c * S:(c + 1) * S] = np.asarray(
            res.results[c]["out"]).astype(np.float32).reshape(S, FOUT, H, W)
    return out


# revision 12
# speedup vs baseline: 1.7460x; 1.7460x over previous
"""MixConv kernel for Trainium2 (Bass/Tile), data-parallel over batch on 8 NeuronCores.

Reference computation (per sample b):
    mix[b]    = lat[b] @ w_dyn.T + b_dyn                      # [NMIX]
    kern[b]   = sum_m mix[b,m] * kernel_mix[m]                # [FOUT, FIN]
    bias[b]   = sum_m mix[b,m] * bias_mix[m]                  # [FOUT]
    out[b]    = kern[b] @ x[b].reshape(FIN, H*W) + bias[b][:, None]

Sharding: batch 16 -> 2 samples per core x 8 cores.

The kernel is HBM-bound (~358 GB/s per NeuronCore when all cores are
active), so the streamed tensors are narrowed on the host (kernel()
receives full fp32; casts are host-side preprocessing): out is written
bf16 and upcast on the host, and x streams as a per-channel precision
split -- the first NB input channels in bf16, the last NF8 in fp8-e3m4.
PSUM accumulation stays fp32 and the per-sample kernel weights stay bf16.
The max-norm error is dominated by worst-case quantization events, so it
plateaus in the fp8 channel count (12/16 fp8 ~= 16/16 fp8 in sim) while
read bytes drop another 37.5% vs all-bf16.  NF8=0 falls back to the pure
bf16 stream (rel-err 4.4e-3; split: ~1.6e-2, gate 2e-2).

The tiny dynamic-weight computation (mix/kern/bias: ~65K FLOPs on 16x512
inputs) is folded on the host into ready-made per-core tensors, so the
device-side setup is just three small DMA loads and the streaming matmuls
start immediately:
  lhsT[p=(s,i,j), q=(s',o,j')] = kern_s[o,i] if s==s' and j==j' else 0
  biasv[q=(s,o,j)] = bias_s[o]
lhsT is row-split to match the channel split; each 512-column chunk is
two accumulating matmuls (bf16 lhsT x bf16 rhs, then bf16 lhsT x fp8 rhs).

Per-core layout (driven by DMA bandwidth: only pure-2D [rows, nt] access
patterns sustain line rate; any 3-dim AP drops to ~100-160GB/s):
  xb viewed as [S*NB*NJ, CHW]  with partition (s, i<NB, j)   (C order)
  x8 viewed as [S*NF8*NJ, CHW] with partition (s, i>=NB, j)  (C order)
  out viewed as [128, CHW]     with partition q = (s, o, j)  (C order)
  One PSUM accumulation group per 512 columns (PSUM bank limit); bias is
  added during the PSUM->SBUF copy (alternating scalar/vector engines)
  which also converts fp32 -> bf16.  x loads ride the sync HWDGE ring,
  out stores the scalar(ACT) ring, 4-deep double-buffered.
"""

import numpy as np
import ml_dtypes

import concourse.bass as bass
import concourse.bacc as bacc
import concourse.tile as tile
import concourse.mybir as mybir
from concourse import bass_utils

B, FIN, FOUT, H, W = 16, 16, 16, 384, 384
LAT, NMIX = 512, 8
N_CORES = 8
S = B // N_CORES          # samples per core = 2
NJ = 4                    # HW chunks per sample
HW = H * W                # 147456
CHW = HW // NJ            # 36864
P = S * NJ * FIN          # 128 partitions
F32 = mybir.dt.float32
BF16 = mybir.dt.bfloat16
E3 = mybir.dt.float8e3

O_DT = BF16               # dtype out is streamed in (host-upcast)
X_NP = ml_dtypes.bfloat16
K_NP = ml_dtypes.bfloat16
E3_NP = ml_dtypes.float8_e3m4

# fp8-e3m4 channel-splitting was measured on HW at rel-err 1.86e-2 (the
# fp8 error does not shrink with fewer fp8 channels on HW, unlike in
# numpy sim) and 106 us (the per-chunk LDWEIGHTS swap between the two
# accumulating matmuls stalls the PE), so the pure-bf16 stream ships.
NF8 = 0                   # input channels streamed as fp8-e3m4 (0 = all bf16)
NB = FIN - NF8            # input channels streamed as bf16

TILES = (9216,) * 4       # stream-tile columns; sums to CHW
assert sum(TILES) == CHW


def host_weights(lat, kernel_mix, bias_mix, w_dyn, b_dyn, nf8=NF8):
    """Per-core row-split block-diagonal lhsT (bf16) and bias vec (f32).

    Returns lists over cores: (lhsT_b [S*(FIN-nf8)*NJ, P], lhsT_8
    [S*nf8*NJ, P], biasv [P, 1]).  lhsT rows follow the (s, i, j) partition
    order of the corresponding x stream; lhsT_8 is None when nf8 == 0.
    """
    nb = FIN - nf8
    mix = lat @ w_dyn.T + b_dyn[None, :]                  # [B, NMIX]
    kern = np.einsum('bm,moi->boi', mix, kernel_mix)      # [B, FOUT, FIN]
    bias = np.einsum('bm,mo->bo', mix, bias_mix)          # [B, FOUT]
    lhsTbs, lhsT8s, biasvs = [], [], []
    i_idx = np.arange(FIN)
    o_idx = np.arange(FOUT)
    for c in range(N_CORES):
        lhsT = np.zeros((P, P), dtype=np.float32)
        biasv = np.zeros((P, 1), dtype=np.float32)
        for s in range(S):
            b = c * S + s
            for j in range(NJ):
                rows = s * FIN * NJ + i_idx * NJ + j      # p = (s, i, j)
                cols = s * FOUT * NJ + o_idx * NJ + j     # q = (s, o, j)
                lhsT[np.ix_(rows, cols)] = kern[b].T      # [i, o]
                biasv[cols, 0] = bias[b]
        lr = lhsT.reshape(S, FIN, NJ, P)
        lhsTbs.append(np.ascontiguousarray(
            lr[:, :nb].reshape(S * nb * NJ, P)).astype(K_NP))
        lhsT8s.append(np.ascontiguousarray(
            lr[:, nb:].reshape(S * nf8 * NJ, P)).astype(K_NP) if nf8 else None)
        biasvs.append(biasv)
    return lhsTbs, lhsT8s, biasvs


def make_in_maps(x, lat, kernel_mix, bias_mix, w_dyn, b_dyn, nf8=NF8):
    """Shard + host-precondition full fp32 inputs into per-core input maps."""
    nb = FIN - nf8
    x = np.ascontiguousarray(np.asarray(x, dtype=np.float32))
    lhsTbs, lhsT8s, biasvs = host_weights(
        np.asarray(lat, dtype=np.float32),
        np.asarray(kernel_mix, dtype=np.float32),
        np.asarray(bias_mix, dtype=np.float32),
        np.asarray(w_dyn, dtype=np.float32),
        np.asarray(b_dyn, dtype=np.float32), nf8=nf8)
    in_maps = []
    for c in range(N_CORES):
        xs = x[c * S:(c + 1) * S].reshape(S, FIN, NJ, CHW)
        m = {
            "xb": np.ascontiguousarray(xs[:, :nb]).astype(X_NP),
            "lhsTb": lhsTbs[c],
            "biasv": biasvs[c],
        }
        if nf8:
            m["x8"] = np.ascontiguousarray(xs[:, nb:]).astype(E3_NP)
            m["lhsT8"] = lhsT8s[c]
        in_maps.append(m)
    return in_maps


def build_nc(s=S, nj=NJ, chw=CHW, tiles=TILES, fin=FIN, fout=FOUT,
             n_cores=N_CORES, loop_repeat=1, mode="full", nf8=NF8,
             xs_bufs=4, os_bufs=4, ps_bufs=4, chunk=512,
             loop_covers_setup=False):
    p = s * nj * fin
    nb = fin - nf8
    pb = s * nb * nj
    p8 = s * nf8 * nj
    assert p <= 128 and sum(tiles) == chw
    assert all(t % chunk == 0 for t in tiles)
    ntmax = max(tiles)

    nc = bacc.Bacc("TRN2", target_bir_lowering=False, debug=False,
                   num_devices=n_cores)
    xb_d = nc.dram_tensor("xb", [s, nb, nj, chw], BF16, kind="ExternalInput").ap()
    lhsTb_d = nc.dram_tensor("lhsTb", [pb, p], BF16, kind="ExternalInput").ap()
    biasv_d = nc.dram_tensor("biasv", [p, 1], F32, kind="ExternalInput").ap()
    if nf8:
        x8_d = nc.dram_tensor("x8", [s, nf8, nj, chw], E3, kind="ExternalInput").ap()
        lhsT8_d = nc.dram_tensor("lhsT8", [p8, p], BF16, kind="ExternalInput").ap()
    out_d = nc.dram_tensor("out", [s, fout, nj, chw], O_DT, kind="ExternalOutput").ap()

    xbf = xb_d.rearrange("s i j c -> (s i j) c")    # [pb, chw], 2D
    x8f = x8_d.rearrange("s i j c -> (s i j) c") if nf8 else None
    of = out_d.rearrange("s o j c -> (s o j) c")    # [p, chw], 2D

    with tile.TileContext(nc) as tc:
        with (
            tc.tile_pool(name="setup", bufs=1) as setup,
            tc.tile_pool(name="xs", bufs=xs_bufs) as xs_pool,
            tc.tile_pool(name="x8s", bufs=xs_bufs) as x8_pool,
            tc.tile_pool(name="os", bufs=os_bufs) as os_pool,
            tc.tile_pool(name="ps", bufs=ps_bufs, space="PSUM") as ps_pool,
        ):
            def emit_setup():
                # Ready-made weights: small loads on the scalar(ACT) ring so
                # the sync ring starts streaming x immediately.
                lhsTb_sb = setup.tile([pb, p], BF16)
                nc.scalar.dma_start(out=lhsTb_sb[:], in_=lhsTb_d[:])
                lhsT8_sb = None
                if nf8:
                    lhsT8_sb = setup.tile([p8, p], BF16)
                    nc.scalar.dma_start(out=lhsT8_sb[:], in_=lhsT8_d[:])
                bias_sb = setup.tile([p, 1], F32)
                nc.scalar.dma_start(out=bias_sb[:], in_=biasv_d[:])
                return lhsTb_sb, lhsT8_sb, bias_sb

            def main_pass(lhsTb_sb, lhsT8_sb, bias_sb):
                col0 = 0
                for nt in tiles:
                    cols = slice(col0, col0 + nt)
                    col0 += nt
                    xbt = xs_pool.tile([pb, ntmax], BF16)
                    x8t = (x8_pool.tile([p8, ntmax], E3, name="x8t")
                           if nf8 else None)
                    if mode != "compute":
                        nc.sync.dma_start(out=xbt[:, :nt], in_=xbf[:, cols])
                        if nf8:
                            nc.sync.dma_start(out=x8t[:, :nt], in_=x8f[:, cols])
                    ot = os_pool.tile([p, ntmax], O_DT)
                    if mode != "dma":
                        for ci in range(nt // chunk):
                            cs = slice(ci * chunk, (ci + 1) * chunk)
                            pt = ps_pool.tile([p, chunk], F32)
                            nc.tensor.matmul(pt[:], lhsTb_sb[:], xbt[:, cs],
                                             start=True, stop=not nf8)
                            if nf8:
                                nc.tensor.matmul(pt[:], lhsT8_sb[:], x8t[:, cs],
                                                 start=False, stop=True)
                            if ci % 2 == 0:
                                nc.scalar.add(ot[:, cs], pt[:], bias_sb[:])
                            else:
                                nc.vector.tensor_scalar_add(ot[:, cs], pt[:],
                                                            bias_sb[:])
                    if mode != "compute":
                        # mode=="dma" stores the (unwritten) ot tile: pure
                        # load+store floor probe without compute deps.
                        nc.scalar.dma_start(out=of[:, cols], in_=ot[:, :nt])

            if loop_repeat > 1 and loop_covers_setup:
                with tc.For_i(0, loop_repeat, 1):
                    args = emit_setup()
                    main_pass(*args)
            elif loop_repeat > 1:
                args = emit_setup()
                with tc.For_i(0, loop_repeat, 1):
                    main_pass(*args)
            else:
                args = emit_setup()
                main_pass(*args)
    nc.compile()
    return nc


_NC = None


def _get_nc():
    global _NC
    if _NC is None:
        _NC = build_nc()
    return _NC


def kernel(x, lat, kernel_mix, bias_mix, w_dyn, b_dyn):
    in_maps = make_in_maps(x, lat, kernel_mix, bias_mix, w_dyn, b_dyn)
    nc = _get_nc()
    res = bass_utils.run_bass_kernel_spmd(nc, in_maps, core_ids=list(range(N_CORES)))
    out = np.empty((B, FOUT, H, W), dtype=np.float32)
    for c in range(N_CORES):
        out[c * S:(c + 1) * S] = np.asarray(
            res.results[c]["out"]).astype(np.float32).reshape(S, FOUT, H, W)
    return out


# revision 17
# speedup vs baseline: 1.8601x; 1.0653x over previous
"""MixConv kernel for Trainium2 (Bass/Tile), data-parallel over batch on 8 NeuronCores.

Reference computation (per sample b):
    mix[b]    = lat[b] @ w_dyn.T + b_dyn                      # [NMIX]
    kern[b]   = sum_m mix[b,m] * kernel_mix[m]                # [FOUT, FIN]
    bias[b]   = sum_m mix[b,m] * bias_mix[m]                  # [FOUT]
    out[b]    = kern[b] @ x[b].reshape(FIN, H*W) + bias[b][:, None]

Sharding: batch 16 -> 2 samples per core x 8 cores.

The kernel is HBM-bound (~358 GB/s per NeuronCore when all cores are
active), so the streamed tensors are narrowed on the host (kernel()
receives full fp32; casts are host-side preprocessing): out is written
bf16 and upcast on the host, and x streams as a per-channel precision
split -- the first NB input channels in bf16, the last NF8 in fp8-e3m4.
PSUM accumulation stays fp32 and the per-sample kernel weights stay bf16.
The max-norm error is dominated by worst-case quantization events, so it
plateaus in the fp8 channel count (12/16 fp8 ~= 16/16 fp8 in sim) while
read bytes drop another 37.5% vs all-bf16.  NF8=0 falls back to the pure
bf16 stream (rel-err 4.4e-3; split: ~1.6e-2, gate 2e-2).

The tiny dynamic-weight computation (mix/kern/bias: ~65K FLOPs on 16x512
inputs) is folded on the host into ready-made per-core tensors, so the
device-side setup is just three small DMA loads and the streaming matmuls
start immediately:
  lhsT[p=(s,i,j), q=(s',o,j')] = kern_s[o,i] if s==s' and j==j' else 0
  biasv[q=(s,o,j)] = bias_s[o]
lhsT is row-split to match the channel split; each 512-column chunk is
two accumulating matmuls (bf16 lhsT x bf16 rhs, then bf16 lhsT x fp8 rhs).

Per-core layout (driven by DMA bandwidth: only pure-2D [rows, nt] access
patterns sustain line rate; any 3-dim AP drops to ~100-160GB/s):
  xb viewed as [S*NB*NJ, CHW]  with partition (s, i<NB, j)   (C order)
  x8 viewed as [S*NF8*NJ, CHW] with partition (s, i>=NB, j)  (C order)
  out viewed as [128, CHW]     with partition q = (s, o, j)  (C order)
  One PSUM accumulation group per 512 columns (PSUM bank limit); bias is
  added during the PSUM->SBUF copy (alternating scalar/vector engines)
  which also converts fp32 -> bf16.  x loads ride the sync HWDGE ring,
  out stores the scalar(ACT) ring, 4-deep double-buffered.
"""

import numpy as np
import ml_dtypes

import concourse.bass as bass
import concourse.bacc as bacc
import concourse.tile as tile
import concourse.mybir as mybir
from concourse import bass_utils

B, FIN, FOUT, H, W = 16, 16, 16, 384, 384
LAT, NMIX = 512, 8
N_CORES = 8
S = B // N_CORES          # samples per core = 2
NJ = 4                    # HW chunks per sample
HW = H * W                # 147456
CHW = HW // NJ            # 36864
P = S * NJ * FIN          # 128 partitions
F32 = mybir.dt.float32
BF16 = mybir.dt.bfloat16
E3 = mybir.dt.float8e3

O_DT = BF16               # dtype out is streamed in (host-upcast)
X_NP = ml_dtypes.bfloat16
K_NP = ml_dtypes.bfloat16
E3_NP = ml_dtypes.float8_e3m4

# fp8-e3m4 channel-splitting was measured on HW at rel-err 1.86e-2 (the
# fp8 error does not shrink with fewer fp8 channels on HW, unlike in
# numpy sim) and 106 us (the per-chunk LDWEIGHTS swap between the two
# accumulating matmuls stalls the PE), so the pure-bf16 stream ships.
NF8 = 0                   # input channels streamed as fp8-e3m4 (0 = all bf16)
NB = FIN - NF8            # input channels streamed as bf16

TILES = (9216,) * 4       # stream-tile columns; sums to CHW
assert sum(TILES) == CHW


def host_weights(lat, kernel_mix, bias_mix, w_dyn, b_dyn, nf8=NF8):
    """Per-core row-split block-diagonal lhsT (bf16) and bias vec (f32).

    Returns lists over cores: (lhsT_b [S*(FIN-nf8)*NJ, P], lhsT_8
    [S*nf8*NJ, P], biasv [P, 1]).  lhsT rows follow the (s, i, j) partition
    order of the corresponding x stream; lhsT_8 is None when nf8 == 0.
    """
    nb = FIN - nf8
    mix = lat @ w_dyn.T + b_dyn[None, :]                  # [B, NMIX]
    kern = np.einsum('bm,moi->boi', mix, kernel_mix)      # [B, FOUT, FIN]
    bias = np.einsum('bm,mo->bo', mix, bias_mix)          # [B, FOUT]
    lhsTbs, lhsT8s, biasvs = [], [], []
    i_idx = np.arange(FIN)
    o_idx = np.arange(FOUT)
    for c in range(N_CORES):
        lhsT = np.zeros((P, P), dtype=np.float32)
        biasv = np.zeros((P, 1), dtype=np.float32)
        for s in range(S):
            b = c * S + s
            for j in range(NJ):
                rows = s * FIN * NJ + i_idx * NJ + j      # p = (s, i, j)
                cols = s * FOUT * NJ + o_idx * NJ + j     # q = (s, o, j)
                lhsT[np.ix_(rows, cols)] = kern[b].T      # [i, o]
                biasv[cols, 0] = bias[b]
        lr = lhsT.reshape(S, FIN, NJ, P)
        lhsTbs.append(np.ascontiguousarray(
            lr[:, :nb].reshape(S * nb * NJ, P)).astype(K_NP))
        lhsT8s.append(np.ascontiguousarray(
            lr[:, nb:].reshape(S * nf8 * NJ, P)).astype(K_NP) if nf8 else None)
        biasvs.append(biasv)
    return lhsTbs, lhsT8s, biasvs


def make_in_maps(x, lat, kernel_mix, bias_mix, w_dyn, b_dyn, nf8=NF8):
    """Shard + host-precondition full fp32 inputs into per-core input maps."""
    nb = FIN - nf8
    x = np.ascontiguousarray(np.asarray(x, dtype=np.float32))
    lhsTbs, lhsT8s, biasvs = host_weights(
        np.asarray(lat, dtype=np.float32),
        np.asarray(kernel_mix, dtype=np.float32),
        np.asarray(bias_mix, dtype=np.float32),
        np.asarray(w_dyn, dtype=np.float32),
        np.asarray(b_dyn, dtype=np.float32), nf8=nf8)
    in_maps = []
    for c in range(N_CORES):
        xs = x[c * S:(c + 1) * S].reshape(S, FIN, NJ, CHW)
        m = {
            "xb": np.ascontiguousarray(xs[:, :nb]).astype(X_NP),
            "lhsTb": lhsTbs[c],
            "biasv": biasvs[c],
        }
        if nf8:
            m["x8"] = np.ascontiguousarray(xs[:, nb:]).astype(E3_NP)
            m["lhsT8"] = lhsT8s[c]
        in_maps.append(m)
    return in_maps


def build_nc(s=S, nj=NJ, chw=CHW, tiles=TILES, fin=FIN, fout=FOUT,
             n_cores=N_CORES, loop_repeat=1, mode="full", nf8=NF8,
             xs_bufs=4, os_bufs=4, ps_bufs=4, chunk=512, store_split=1,
             store_rings=("scalar",), loop_covers_setup=False):
    p = s * nj * fin
    nb = fin - nf8
    pb = s * nb * nj
    p8 = s * nf8 * nj
    assert p <= 128 and sum(tiles) == chw
    assert all(t % chunk == 0 for t in tiles)
    ntmax = max(tiles)

    nc = bacc.Bacc("TRN2", target_bir_lowering=False, debug=False,
                   num_devices=n_cores)
    xb_d = nc.dram_tensor("xb", [s, nb, nj, chw], BF16, kind="ExternalInput").ap()
    lhsTb_d = nc.dram_tensor("lhsTb", [pb, p], BF16, kind="ExternalInput").ap()
    biasv_d = nc.dram_tensor("biasv", [p, 1], F32, kind="ExternalInput").ap()
    if nf8:
        x8_d = nc.dram_tensor("x8", [s, nf8, nj, chw], E3, kind="ExternalInput").ap()
        lhsT8_d = nc.dram_tensor("lhsT8", [p8, p], BF16, kind="ExternalInput").ap()
    out_d = nc.dram_tensor("out", [s, fout, nj, chw], O_DT, kind="ExternalOutput").ap()

    xbf = xb_d.rearrange("s i j c -> (s i j) c")    # [pb, chw], 2D
    x8f = x8_d.rearrange("s i j c -> (s i j) c") if nf8 else None
    of = out_d.rearrange("s o j c -> (s o j) c")    # [p, chw], 2D

    with tile.TileContext(nc) as tc:
        with (
            tc.tile_pool(name="setup", bufs=1) as setup,
            tc.tile_pool(name="xs", bufs=xs_bufs) as xs_pool,
            tc.tile_pool(name="x8s", bufs=xs_bufs) as x8_pool,
            tc.tile_pool(name="os", bufs=os_bufs) as os_pool,
            tc.tile_pool(name="ps", bufs=ps_bufs, space="PSUM") as ps_pool,
        ):
            def emit_setup():
                # Ready-made weights: small loads on the scalar(ACT) ring so
                # the sync ring starts streaming x immediately.
                lhsTb_sb = setup.tile([pb, p], BF16)
                nc.scalar.dma_start(out=lhsTb_sb[:], in_=lhsTb_d[:])
                lhsT8_sb = None
                if nf8:
                    lhsT8_sb = setup.tile([p8, p], BF16)
                    nc.scalar.dma_start(out=lhsT8_sb[:], in_=lhsT8_d[:])
                bias_sb = setup.tile([p, 1], F32)
                nc.scalar.dma_start(out=bias_sb[:], in_=biasv_d[:])
                return lhsTb_sb, lhsT8_sb, bias_sb

            def main_pass(lhsTb_sb, lhsT8_sb, bias_sb):
                store_engines = [getattr(nc, nm) for nm in store_rings]
                col0 = 0
                for ti, nt in enumerate(tiles):
                    cols = slice(col0, col0 + nt)
                    col0 += nt
                    xbt = xs_pool.tile([pb, ntmax], BF16)
                    x8t = (x8_pool.tile([p8, ntmax], E3, name="x8t")
                           if nf8 else None)
                    if mode != "compute":
                        nc.sync.dma_start(out=xbt[:, :nt], in_=xbf[:, cols])
                        if nf8:
                            nc.sync.dma_start(out=x8t[:, :nt], in_=x8f[:, cols])
                    ot = os_pool.tile([p, ntmax], O_DT)
                    if mode != "dma":
                        for ci in range(nt // chunk):
                            cs = slice(ci * chunk, (ci + 1) * chunk)
                            pt = ps_pool.tile([p, chunk], F32)
                            nc.tensor.matmul(pt[:], lhsTb_sb[:], xbt[:, cs],
                                             start=True, stop=not nf8)
                            if nf8:
                                nc.tensor.matmul(pt[:], lhsT8_sb[:], x8t[:, cs],
                                                 start=False, stop=True)
                            if ci % 2 == 0:
                                nc.scalar.add(ot[:, cs], pt[:], bias_sb[:])
                            else:
                                nc.vector.tensor_scalar_add(ot[:, cs], pt[:],
                                                            bias_sb[:])
                    if mode != "compute":
                        # mode=="dma" stores the (unwritten) ot tile: pure
                        # load+store floor probe without compute deps.
                        # store_split>1 issues the store in column slices so
                        # the final store need not wait for the whole tile's
                        # adds (shorter post-last-load drain).
                        hs = nt // store_split
                        for h in range(store_split):
                            store_engines[ti % len(store_engines)].dma_start(
                                out=of[:, cols.start + h * hs:
                                       cols.start + (h + 1) * hs],
                                in_=ot[:, h * hs:(h + 1) * hs])

            if loop_repeat > 1 and loop_covers_setup:
                with tc.For_i(0, loop_repeat, 1):
                    args = emit_setup()
                    main_pass(*args)
            elif loop_repeat > 1:
                args = emit_setup()
                with tc.For_i(0, loop_repeat, 1):
                    main_pass(*args)
            else:
                args = emit_setup()
                main_pass(*args)
    nc.compile()
    return nc


_NC = None


def _get_nc():
    global _NC
    if _NC is None:
        _NC = build_nc()
    return _NC


def kernel(x, lat, kernel_mix, bias_mix, w_dyn, b_dyn):
    in_maps = make_in_maps(x, lat, kernel_mix, bias_mix, w_dyn, b_dyn)
    nc = _get_nc()
    res = bass_utils.run_bass_kernel_spmd(nc, in_maps, core_ids=list(range(N_CORES)))
    out = np.empty((B, FOUT, H, W), dtype=np.float32)
    for c in range(N_CORES):
        out[c * S:(c + 1) * S] = np.asarray(
            res.results[c]["out"]).astype(np.float32).reshape(S, FOUT, H, W)
    return out
